# revision 54
# baseline (speedup 1.0000x reference)
"""Causal self-attention with rotary embeddings on 8 Trainium2 NeuronCores.

Tensor-parallel over heads: 16 heads / 8 cores = 2 heads per core.
Each core computes qkv for its 2 heads, rotary, causal attention, and a
partial output projection (its 128 rows of w_proj); the host sums the 8
partial outputs.

Device-side structure (per core, heads A/B local):
  - Everything "transposed": Q^T/K^T stored [d(128=A:0-63,B:64-127), t].
  - Work is emitted as a pipeline of (batch, q-chunk) units:
      scores+exp(unit) | qkv+rotary(next chunk) | PV | normalize | proj
    so TensorE always has dense matmul work while ScalarE exponentiates
    and the projection + y DMA spread across the whole kernel.
  - Scores S^T = K_blk @ Q^T -> [k(128), q], computed for both heads as
    two row-tiled matmuls (head A rows 0-63, head B rows 64-127) that
    run concurrently in the PE array (K=64 each).
  - exp per k-block over both heads in one ACTIVATE on [128, 2, W-off]
    (off = causal column offset); diagonal 128-col block masked via one
    GpSimd multiply (2-head mask const); PV matmuls use partial-N
    accumulation so fully-masked columns are never touched.
  - Softmax denominator via a ones-augmented V column (extra lhsT column
    produces the k-sum row). No max-subtraction (scores are O(6)).
  - Rotary applied in the transposed layout via a pair-swap permutation
    matmul: rot(q) = cos_exp * q + sin_sgn * (Pswap @ q).
  - V transposed to t-major [k, d] tiles with the PE transpose path.
  - The last unit is split into two 256-wide half-units so its
    normalize->proj->DMA tail pipelines instead of serializing.

DMA layout: three rings (sync HWDGE, scalar HWDGE, gpsimd SWDGE q0).
x chunks are split half/half across sync+gpsimd and prefetched one unit
early; startup constants are wave-ordered by first-use time so the first
QKV matmuls start ~9us in and the PE never idles past the HAM window.

All matmul inputs fp16 (1 cyc/row on PE); accumulation fp32 in PSUM.
"""

import numpy as np

B, T, C, H = 2, 2048, 1024, 16
HD = C // H            # 64
N_CORES = 8
HPC = H // N_CORES     # 2 heads per core
BT = B * T             # 4096
TC = 512               # t-chunk size for qkv phase
NC_ = T // TC          # 4 chunks per batch
KB = 128               # k-block size
NKB = T // KB          # 16 k-blocks per batch
CCH = C // 128         # 8 contraction chunks

_CACHE = {}


def _build_bass():
    import concourse.bacc as bacc
    import concourse.mybir as mybir
    import concourse.tile as tile
    from concourse.masks import make_identity, make_upper_triangular

    f16 = mybir.dt.float16
    f32 = mybir.dt.float32

    nc = bacc.Bacc(num_swdge_queues=4)

    # host-prepacked layouts: per-partition-contiguous so DMA bursts are
    # large (8KB x chunks, 2KB wqkv group slices) instead of sub-KB rows
    xT = nc.dram_tensor("xT", [128, NC_ * B, CCH, TC], f16,
                        kind="ExternalInput")
    wqkv = nc.dram_tensor("wqkv", [128, 3, CCH, 128], f16,
                          kind="ExternalInput")
    wp = nc.dram_tensor("wp", [HPC * HD, C], f16, kind="ExternalInput")
    cos_e = nc.dram_tensor("cos_e", [128, T], f16, kind="ExternalInput")
    sin_e = nc.dram_tensor("sin_e", [128, T], f16, kind="ExternalInput")
    pswap = nc.dram_tensor("pswap", [128, 128], f16, kind="ExternalInput")
    y = nc.dram_tensor("y", [BT, C], f16, kind="ExternalOutput")

    with tile.TileContext(nc) as tc:
        with (
            tc.tile_pool(name="const", bufs=1) as const,
            tc.tile_pool(name="persist", bufs=1) as persist,
            tc.tile_pool(name="xp", bufs=4) as xp,
            tc.tile_pool(name="rot", bufs=3) as rotp,
            tc.tile_pool(name="ptp", bufs=24) as ptp,
            tc.tile_pool(name="np_", bufs=3) as normp,
            tc.tile_pool(name="yp", bufs=2) as yp,
            tc.tile_pool(name="work", bufs=2, space="PSUM") as work,
            tc.tile_pool(name="acc2", bufs=2, space="PSUM") as acc2,
            tc.tile_pool(name="stp", bufs=2, space="PSUM") as stp,
        ):
            # ---- constants: wave-ordered startup DMAs across 3 rings ----
            # scalar ring carries the weights (ordered by first use);
            # sync/gpsimd carry the first x chunk + first cos/sin columns.
            wqkv_sb = const.tile([128, 3, CCH, 128], f16)
            cos_sb = const.tile([128, T], f16)
            sin_sb = const.tile([128, T], f16)
            pswap_sb = const.tile([128, 128], f16)
            wp_sb = const.tile([128, C], f16)

            nc.scalar.dma_start(out=wqkv_sb[:, 0], in_=wqkv[:, 0])  # Q
            nc.scalar.dma_start(out=wqkv_sb[:, 1], in_=wqkv[:, 1])  # K
            nc.scalar.dma_start(out=pswap_sb, in_=pswap[:, :])
            nc.scalar.dma_start(out=wqkv_sb[:, 2], in_=wqkv[:, 2])  # V
            nc.scalar.dma_start(out=wp_sb, in_=wp[:, :])
            nc.scalar.dma_start(out=cos_sb[:, TC:], in_=cos_e[:, TC:])
            nc.scalar.dma_start(out=sin_sb[:, TC:], in_=sin_e[:, TC:])
            # head-B projection rows re-staged at partitions 0-63 for the
            # tail half-units (lets their proj read ytmp directly and skip
            # the cross-partition YnB DMA); only needed at ~170us
            wpB_sb = const.tile([64, C], f16)
            nc.scalar.dma_start(out=wpB_sb, in_=wp[64:128, :])

            # x chunk 0 before anything else on the sync/gpsimd rings
            # (ring FIFO = priority), then the chunk-0 cos/sin columns,
            # then the chunk-1 prefetch rides behind them.
            x0_sb = xp.tile([128, CCH, TC], f16, tag="x")
            nc.sync.dma_start(out=x0_sb[:, 0:2, :], in_=xT[:, 0, 0:2, :])
            nc.sync.dma_start(out=x0_sb[:, 2:4, :], in_=xT[:, 0, 2:4, :])
            nc.gpsimd.dma_start(out=x0_sb[:, 4:6, :], in_=xT[:, 0, 4:6, :])
            nc.gpsimd.dma_start(out=x0_sb[:, 6:8, :], in_=xT[:, 0, 6:8, :])
            nc.sync.dma_start(out=sin_sb[:, 0:TC], in_=sin_e[:, 0:TC])
            nc.gpsimd.dma_start(out=cos_sb[:, 0:TC], in_=cos_e[:, 0:TC])

            ident = const.tile([128, 128], f16)
            make_identity(nc, ident)
            # mask2[k, h, q] = 1 where q >= k (keep), both heads
            mask2 = const.tile([128, 2, 128], f16)
            make_upper_triangular(nc, mask2[:, 0, :], val=1.0, diag=True)
            make_upper_triangular(nc, mask2[:, 1, :], val=1.0, diag=True)
            # row of ones: lhsT of the reciprocal-broadcast matmul
            ones64 = const.tile([1, 64], f16)
            nc.gpsimd.memset(ones64, 1.0)

            # PE warmup: dependency-free matmuls on the on-chip identity
            # bridge the PE HAM activity window until the first x data
            # lands (~9.5us), so real matmuls start at 2.4 GHz.
            warm_ps = work.tile([128, 128], f32, tag="work", name="warm")
            for _ in range(18):
                nc.tensor.matmul(warm_ps, ident, ident,
                                 start=True, stop=True)

            # ---- persistent tensors ----
            QrotT = persist.tile([128, B, T], f16)
            KrotT = persist.tile([128, B, T], f16)
            # V in t-major, per (batch, k-block): [V_A(64) | ones | V_B(64) | ones]
            Vaug = persist.tile([128, B, NKB, 130], f16)
            Yn = persist.tile([128, B, T], f16)
            ones_cols = Vaug.rearrange(
                "p b J (h x) -> p b J h x", x=65)[:, :, :, :, 64]
            nc.gpsimd.memset(ones_cols, 1.0)

            # ============ phase-1: x DMA (separate) + qkv compute ==========
            def dma_x(ci):
                """Issue chunk ci's x load, split across sync+gpsimd."""
                x_sb = xp.tile([128, CCH, TC], f16, tag="x")
                nc.sync.dma_start(out=x_sb[:, 0:4, :],
                                  in_=xT[:, ci, 0:4, :])
                nc.gpsimd.dma_start(out=x_sb[:, 4:8, :],
                                    in_=xT[:, ci, 4:8, :])
                return x_sb

            x_tiles = {}

            def ph1_compute(b, i, swp_pool=None):
                """Emission closures for qkv+rotary of t-chunk i, batch b."""
                ci = b * NC_ + i
                cs = slice(i * TC, (i + 1) * TC)
                state = {}
                spool = swp_pool or work
                stag = "st" if swp_pool is not None else "work"

                def qk_group(g):
                    x_sb = x_tiles[ci]
                    dst = QrotT if g == 0 else KrotT
                    acc = work.tile([128, TC], f32, tag="work", name="acc")
                    for cc in range(CCH):
                        nc.tensor.matmul(
                            acc, wqkv_sb[:, g, cc, :], x_sb[:, cc, :],
                            start=(cc == 0), stop=(cc == CCH - 1))
                    graw = rotp.tile([128, TC], f16, tag="graw")
                    nc.vector.tensor_copy(graw, acc)
                    swp = spool.tile([128, TC], f32, tag=stag, name="swp")
                    nc.tensor.matmul(swp, pswap_sb, graw,
                                     start=True, stop=True)
                    t1 = rotp.tile([128, TC], f16, tag="t1")
                    nc.vector.tensor_mul(t1, graw, cos_sb[:, cs])
                    t2 = rotp.tile([128, TC], f16, tag="t2")
                    nc.vector.tensor_mul(t2, swp, sin_sb[:, cs])
                    nc.vector.tensor_add(dst[:, b, cs], t1, t2)

                def v_group():
                    x_sb = x_tiles[ci]
                    acc = work.tile([128, TC], f32, tag="work", name="vacc")
                    for cc in range(CCH):
                        nc.tensor.matmul(
                            acc, wqkv_sb[:, 2, cc, :], x_sb[:, cc, :],
                            start=(cc == 0), stop=(cc == CCH - 1))
                    vtmp = rotp.tile([128, TC], f16, tag="vtmp")
                    nc.vector.tensor_copy(vtmp, acc)
                    state["vtmp"] = vtmp

                def v_trans():
                    vtmp = state["vtmp"]
                    for q in range(TC // 128):
                        J = i * (TC // 128) + q
                        vt = work.tile([128, 128], f16, tag="work", name="vt")
                        nc.tensor.transpose(
                            vt, vtmp[:, q * 128:(q + 1) * 128], ident)
                        vdst = Vaug[:, b, J, :].rearrange(
                            "p (h x) -> p h x", x=65)[:, :, 0:64]
                        vsrc = vt.rearrange("p (h x) -> p h x", h=2)
                        nc.vector.tensor_copy(vdst, vsrc)

                return [lambda: qk_group(0), lambda: qk_group(1),
                        v_group, v_trans]

            # ============ phase-2 unit: attention for (b, q-window) ========
            def emit_unit(b, qs, W, next_pieces, last, n_act_pout):
                """Attention + normalize + proj for queries [qs, qs+W)."""
                jmax = (qs + W) // KB - 1
                # for the tail halves, process diagonal (masked) blocks
                # first so the final scores->exp->mask->PV chain is a
                # mask-free full block
                js = list(range(jmax + 1))
                pts = {}
                pieces_done = 0
                # tiles are allocated full-width (uniform slot sizes per
                # pool tag) and sliced to W
                ypss = [acc2.tile([128, TC], f32, tag="acc2", name="yps")
                        for _ in range(2)]

                def pv(j):
                    pt, off = pts[j]
                    for h in range(2):
                        nc.tensor.matmul(
                            ypss[h][0:65, off:W],
                            Vaug[:, b, j, h * 65:(h + 1) * 65],
                            pt[:, h, off:W],
                            start=(j == js[0]), stop=(j == js[-1]))

                for ji, j in enumerate(js):
                    off = max(0, j * KB - qs)  # valid col offset in window
                    stf = stp.tile([128, 2, TC], f32, tag="st", name="st")
                    st = stf[:, :, 0:W]
                    for h in range(2):
                        hs = slice(h * 64, (h + 1) * 64)
                        nc.tensor.matmul(
                            st[:, h, off:W],
                            KrotT[hs, b, j * KB:(j + 1) * KB],
                            QrotT[hs, b, qs + off:qs + W],
                            start=True, stop=True)
                    ptf = ptp.tile([128, 2, TC], f16, tag="pt", name="pt")
                    pt = ptf[:, :, 0:W]
                    nc.scalar.activation(
                        pt[:, :, off:W], st[:, :, off:W],
                        mybir.ActivationFunctionType.Exp)
                    if j * KB >= qs:  # diagonal band: triangular mask
                        nc.vector.tensor_mul(
                            pt[:, :, off:off + 128],
                            pt[:, :, off:off + 128], mask2)
                    pts[j] = (pt, off)
                    # PV trails scores by 2 so exp/mask have drained
                    if ji >= 2:
                        pv(js[ji - 2])
                    # interleave next chunk's qkv work into the PE stream
                    want = (len(next_pieces) * (ji + 1)) // (jmax + 1)
                    while pieces_done < want:
                        next_pieces[pieces_done]()
                        pieces_done += 1
                while pieces_done < len(next_pieces):
                    next_pieces[pieces_done]()
                    pieces_done += 1
                pv(js[-2])
                pv(js[-1])
                if last == 2:
                    # tail filler, data-dependent on the last block's pt so
                    # the scheduler can't hoist it: keeps the PE HAM-warm
                    # through the final normalize chain
                    tail_ps = work.tile([128, 128], f32, tag="work",
                                        name="tail_ps")
                    lastpt = pts[js[-1]][0]
                    for _ in range(12):
                        nc.tensor.matmul(tail_ps, ident,
                                         lastpt[:, 0, 0:128],
                                         start=True, stop=True)

                # ---- normalize: rows 0-63 divided by the ones-row (64) ----
                # custom-DVE reciprocal misreads PSUM/cross-partition inputs,
                # so stage both heads' denominators into SBUF partition 0.
                dsbf = normp.tile([1, 2, TC], f32, tag="dsb")
                dsb = dsbf[:, :, 0:W]
                for h in range(2):
                    nc.vector.tensor_copy(dsb[0:1, h, :],
                                          ypss[h][64:65, 0:W])
                recf = normp.tile([1, 2, TC], f32, tag="rec")
                rec = recf[:, :, 0:W]
                for h in range(2):
                    nc.vector.reciprocal_approx_fast(
                        out=rec[0:1, h, :], in_=dsb[0:1, h, :])
                bcf = normp.tile([64, 2, TC], f32, tag="bc", name="bc")
                bc = bcf[:, :, 0:W]
                for h in range(2):
                    nc.gpsimd.partition_broadcast(bc[:, h, :], rec[0:1, h, :])
                cslice = slice(qs, qs + W)
                nc.vector.tensor_tensor(
                    out=Yn[0:64, b, cslice],
                    in0=ypss[0][0:64, 0:W], in1=bc[:, 0, :],
                    op=mybir.AluOpType.mult)
                ytmpf = normp.tile([64, TC], f16, tag="ytmp")
                ytmp = ytmpf[:, 0:W]
                nc.vector.tensor_tensor(
                    out=ytmp, in0=ypss[1][0:64, 0:W], in1=bc[:, 1, :],
                    op=mybir.AluOpType.mult)
                if not last:
                    # cross-partition move 0-63 -> 64-127 via DMA
                    nc.scalar.dma_start(out=Yn[64:128, b, cslice], in_=ytmp)

                # ---- projection for this q-window ----
                r0 = b * T + qs
                ntt = W // 128
                youtf = yp.tile([128, TC // 128, C], f16, tag="yo",
                                name="yout")
                yout = youtf[:, 0:ntt, :]
                for tt in range(ntt):
                    ts = slice(qs + tt * 128, qs + (tt + 1) * 128)
                    for half in range(2):
                        hw = slice(half * 512, (half + 1) * 512)
                        pout = acc2.tile([128, 512], f32, tag="acc2",
                                         name="pout")
                        if last:
                            # head B read straight from ytmp (partitions
                            # 0-63): no cross-partition YnB DMA in the
                            # tail chain, at the cost of a 2nd K=64 pass
                            nc.tensor.matmul(
                                pout, Yn[0:64, b, ts], wp_sb[0:64, hw],
                                start=True, stop=False)
                            nc.tensor.matmul(
                                pout, ytmp[:, tt * 128:(tt + 1) * 128],
                                wpB_sb[:, hw],
                                start=False, stop=True)
                        else:
                            nc.tensor.matmul(
                                pout, Yn[:, b, ts], wp_sb[:, hw],
                                start=True, stop=True)
                        dstap = yout[:, tt, half * 512:(half + 1) * 512]
                        if last == 2:
                            # final half: split each copy across ACT+DVE
                            # so the drain isn't paced by one engine
                            nc.scalar.copy(dstap[:, 0:256], pout[:, 0:256])
                            nc.vector.tensor_copy(dstap[:, 256:512],
                                                  pout[:, 256:512])
                        elif (tt * 2 + half) >= 2 * ntt - n_act_pout:
                            nc.scalar.copy(dstap, pout)
                        else:
                            nc.vector.tensor_copy(dstap, pout)
                    if last:
                        eng = (nc.gpsimd, nc.sync, nc.scalar, nc.sync)[tt % 4]
                        eng.dma_start(
                            out=y[r0 + tt * 128:r0 + (tt + 1) * 128, :],
                            in_=yout[:, tt, :])
                if not last:
                    yeng = nc.gpsimd if (qs // TC + b) % 2 == 0 else nc.scalar
                    yeng.dma_start(
                        out=y[r0:r0 + W, :].rearrange(
                            "(tt p) c -> p tt c", p=128),
                        in_=yout)

            # ================= emission: pipelined units ===================
            # units: chunks 0-6 full-width; chunk 7 as two 256 halves so
            # the tail chain pipelines.
            # each entry: (b, qs, W); chunk feeds are assigned to units in
            # order: unit k issues dma_x(k+2) and computes chunk k+1 where
            # k counts FULL-chunk positions (splits share their chunk's k).
            split_ci = 3 + NC_            # chunk index to emit as 2 halves
            units = []
            for ci in range(8):
                bb, ii = ci // NC_, ci % NC_
                if ci == split_ci:
                    units.append((bb, ii * TC, 256, ci))
                    units.append((bb, ii * TC + 256, 256, ci))
                else:
                    units.append((bb, ii * TC, TC, ci))

            x_tiles[0] = x0_sb
            x_tiles[1] = dma_x(1)
            x_tiles[2] = dma_x(2)
            # dummy broadcast AFTER the x prefetch issues: forces the
            # gpsimd custom-op library load to start now (its TDRAM DMA
            # rides behind x0-x2 on the ring, done ~24us) without
            # blocking any x-chunk DMA issue; unit 0's first real
            # broadcast (~31us) then never stalls on the load.
            libwarm = const.tile([64, 8], f32)
            nc.gpsimd.partition_broadcast(libwarm, libwarm[0:1, :])
            # chunk 0: Q/K groups now (swp staged via the idle stp pool so
            # the K chain isn't WAR-serialized on the 2-slot work ring);
            # V + transpose deferred into unit 0's feed.
            p0 = ph1_compute(0, 0, swp_pool=stp)
            p0[0]()
            p0[1]()
            fed_dma = {0, 1, 2}
            fed_cmp = {0}
            extra0 = [p0[2], p0[3]]
            for u, (b, qs, W, ci) in enumerate(units):
                nxt = list(extra0)
                extra0 = []
                if ci + 3 <= 7 and ci + 3 not in fed_dma:
                    fed_dma.add(ci + 3)
                    nxt.append(lambda c3=ci + 3: x_tiles.__setitem__(
                        c3, dma_x(c3)))
                if ci + 1 <= 7 and ci + 1 not in fed_cmp:
                    fed_cmp.add(ci + 1)
                    nxt.extend(ph1_compute((ci + 1) // NC_, (ci + 1) % NC_))
                last = 2 if u == len(units) - 1 else 0
                if ci == split_ci:
                    last = max(last, 1)
                n_act = (2 if last else
                         {0: 4, 1: 2, 2: 0, 3: 0}[min(qs // TC, 3)])
                emit_unit(b, qs, W, nxt, last, n_act)

    nc.finalize()
    return nc


def _host_prep(x, cos, sin, w_attn, b_attn, w_proj):
    """Shared + per-core input arrays (all fp16 except noted)."""
    x2 = np.asarray(x, dtype=np.float32).reshape(BT, C)
    xT16 = np.ascontiguousarray(x2.T).astype(np.float16)
    # repack [C, BT] -> [p, chunk, cc, t] so each DMA chunk reads one
    # contiguous 8KB run per partition
    xprep = np.ascontiguousarray(
        xT16.reshape(CCH, 128, B * NC_, TC).transpose(1, 2, 0, 3))

    cos = np.asarray(cos, dtype=np.float32)
    sin = np.asarray(sin, dtype=np.float32)
    d = np.arange(128) % 64
    freq_i = d // 2
    sign = np.where(d % 2 == 0, -1.0, 1.0).astype(np.float32)
    cos_exp = cos[:, freq_i].T.astype(np.float16)               # [128, T]
    sin_exp = (sign[:, None] * sin[:, freq_i].T).astype(np.float16)

    pswap = np.zeros((128, 128), dtype=np.float16)
    idx = np.arange(128)
    pswap[idx ^ 1, idx] = 1.0

    w_attn = np.asarray(w_attn, dtype=np.float32)
    w_proj = np.asarray(w_proj, dtype=np.float32)
    scale = 1.0 / np.sqrt(HD)

    per_core = []
    for m in range(N_CORES):
        groups = []
        for g in range(3):          # q, k, v blocks of w_attn
            cols = []
            for hh in range(HPC):
                hglob = m * HPC + hh
                blk = w_attn[:, g * C + hglob * HD:(g * C + (hglob + 1) * HD)]
                if g == 0:
                    blk = blk * scale
                cols.append(blk)
            gs = np.concatenate(cols, axis=1).astype(np.float16)  # [C, 128]
            # [C, 128] -> [p, cc, j] (contiguous 2KB per partition)
            groups.append(gs.reshape(CCH, 128, 128).transpose(1, 0, 2))
        w_stack = np.ascontiguousarray(np.stack(groups, axis=1))
        wp_m = w_proj[m * HPC * HD:(m + 1) * HPC * HD, :].astype(np.float16)
        per_core.append((w_stack, wp_m))
    return xprep, cos_exp, sin_exp, pswap, per_core


def kernel(x, cos, sin, w_attn, b_attn, w_proj, b_proj):
    from concourse.bass_utils import run_bass_kernel_spmd

    b_attn = np.asarray(b_attn, dtype=np.float32)
    assert not np.any(b_attn), "nonzero b_attn not supported by this kernel"

    xT16, cos_exp, sin_exp, pswap, per_core = _host_prep(
        x, cos, sin, w_attn, b_attn, w_proj)

    if "nc" not in _CACHE:
        _CACHE["nc"] = _build_bass()
    nc = _CACHE["nc"]

    in_maps = []
    for m in range(N_CORES):
        w_stack, wp_m = per_core[m]
        in_maps.append({
            "xT": xT16, "wqkv": w_stack, "wp": wp_m,
            "cos_e": cos_exp, "sin_e": sin_exp, "pswap": pswap,
        })

    res = run_bass_kernel_spmd(nc, in_maps, core_ids=list(range(N_CORES)))
    _CACHE["last_result"] = res

    y = np.zeros((BT, C), dtype=np.float64)
    for m in range(N_CORES):
        y += res.results[m]["y"].astype(np.float64)
    y = y + np.asarray(b_proj, dtype=np.float64)[None, :]
    return y.reshape(B, T, C).astype(np.float32)


# revision 55
# speedup vs baseline: 1.2016x; 1.2016x over previous
"""Causal self-attention with rotary embeddings on 8 Trainium2 NeuronCores.

Tensor-parallel over heads: 16 heads / 8 cores = 2 heads per core.
Each core computes qkv for its 2 heads, rotary, causal attention, and a
partial output projection (its 128 rows of w_proj); the host sums the 8
partial outputs.

Device-side structure (per core, heads A/B local):
  - Everything "transposed": Q^T/K^T stored [d(128=A:0-63,B:64-127), t].
  - Work is emitted as a pipeline of (batch, q-chunk) units:
      scores+exp(unit) | qkv+rotary(next chunk) | PV | normalize | proj
    so TensorE always has dense matmul work while ScalarE exponentiates
    and the projection + y DMA spread across the whole kernel.
  - Scores S^T = K_blk @ Q^T -> [k(128), q], computed for both heads as
    two row-tiled matmuls (head A rows 0-63, head B rows 64-127) that
    run concurrently in the PE array (K=64 each).
  - exp per k-block over both heads in one ACTIVATE on [128, 2, W-off]
    (off = causal column offset); diagonal 128-col block masked via one
    GpSimd multiply (2-head mask const); PV matmuls use partial-N
    accumulation so fully-masked columns are never touched.
  - Softmax denominator via a ones-augmented V column (extra lhsT column
    produces the k-sum row). No max-subtraction (scores are O(6)).
  - Rotary applied in the transposed layout via a pair-swap permutation
    matmul: rot(q) = cos_exp * q + sin_sgn * (Pswap @ q).
  - V transposed to t-major [k, d] tiles with the PE transpose path.
  - The last unit is split into two 256-wide half-units so its
    normalize->proj->DMA tail pipelines instead of serializing.

DMA layout: three rings (sync HWDGE, scalar HWDGE, gpsimd SWDGE q0).
x chunks are split half/half across sync+gpsimd and prefetched one unit
early; startup constants are wave-ordered by first-use time so the first
QKV matmuls start ~9us in and the PE never idles past the HAM window.

All matmul inputs fp16 (1 cyc/row on PE); accumulation fp32 in PSUM.
"""

import numpy as np

B, T, C, H = 2, 2048, 1024, 16
HD = C // H            # 64
N_CORES = 8
HPC = H // N_CORES     # 2 heads per core
BT = B * T             # 4096
TC = 512               # t-chunk size for qkv phase
NC_ = T // TC          # 4 chunks per batch
KB = 128               # k-block size
NKB = T // KB          # 16 k-blocks per batch
CCH = C // 128         # 8 contraction chunks

_CACHE = {}


def _build_bass():
    import concourse.bacc as bacc
    import concourse.mybir as mybir
    import concourse.tile as tile
    from concourse.masks import make_identity, make_upper_triangular

    f16 = mybir.dt.float16
    f32 = mybir.dt.float32

    nc = bacc.Bacc(num_swdge_queues=4)

    # host-prepacked layouts: per-partition-contiguous so DMA bursts are
    # large (8KB x chunks, 2KB wqkv group slices) instead of sub-KB rows
    xT = nc.dram_tensor("xT", [128, NC_ * B, CCH, TC], f16,
                        kind="ExternalInput")
    wqkv = nc.dram_tensor("wqkv", [128, 3, CCH, 128], f16,
                          kind="ExternalInput")
    wp = nc.dram_tensor("wp", [HPC * HD, C], f16, kind="ExternalInput")
    cos_e = nc.dram_tensor("cos_e", [128, T], f16, kind="ExternalInput")
    sin_e = nc.dram_tensor("sin_e", [128, T], f16, kind="ExternalInput")
    pswap = nc.dram_tensor("pswap", [128, 128], f16, kind="ExternalInput")
    y = nc.dram_tensor("y", [BT, C], f16, kind="ExternalOutput")

    with tile.TileContext(nc) as tc:
        with (
            tc.tile_pool(name="const", bufs=1) as const,
            tc.tile_pool(name="persist", bufs=1) as persist,
            tc.tile_pool(name="xp", bufs=4) as xp,
            tc.tile_pool(name="rot", bufs=3) as rotp,
            tc.tile_pool(name="ptp", bufs=24) as ptp,
            tc.tile_pool(name="np_", bufs=3) as normp,
            tc.tile_pool(name="yp", bufs=2) as yp,
            tc.tile_pool(name="work", bufs=2, space="PSUM") as work,
            tc.tile_pool(name="acc2", bufs=2, space="PSUM") as acc2,
            tc.tile_pool(name="stp", bufs=2, space="PSUM") as stp,
        ):
            # ---- constants: wave-ordered startup DMAs across 3 rings ----
            # scalar ring carries the weights (ordered by first use);
            # sync/gpsimd carry the first x chunk + first cos/sin columns.
            wqkv_sb = const.tile([128, 3, CCH, 128], f16)
            cos_sb = const.tile([128, T], f16)
            sin_sb = const.tile([128, T], f16)
            pswap_sb = const.tile([128, 128], f16)
            wp_sb = const.tile([128, C], f16)

            nc.scalar.dma_start(out=wqkv_sb[:, 0], in_=wqkv[:, 0])  # Q
            nc.scalar.dma_start(out=wqkv_sb[:, 1], in_=wqkv[:, 1])  # K
            nc.scalar.dma_start(out=pswap_sb, in_=pswap[:, :])
            nc.scalar.dma_start(out=wqkv_sb[:, 2], in_=wqkv[:, 2])  # V
            nc.scalar.dma_start(out=wp_sb, in_=wp[:, :])
            nc.scalar.dma_start(out=cos_sb[:, TC:], in_=cos_e[:, TC:])
            nc.scalar.dma_start(out=sin_sb[:, TC:], in_=sin_e[:, TC:])
            # head-B projection rows re-staged at partitions 0-63 for the
            # tail half-units (lets their proj read ytmp directly and skip
            # the cross-partition YnB DMA); only needed at ~170us
            wpB_sb = const.tile([64, C], f16)
            nc.scalar.dma_start(out=wpB_sb, in_=wp[64:128, :])

            # x chunk 0 before anything else on the sync/gpsimd rings
            # (ring FIFO = priority), then the chunk-0 cos/sin columns,
            # then the chunk-1 prefetch rides behind them.
            x0_sb = xp.tile([128, CCH, TC], f16, tag="x")
            nc.sync.dma_start(out=x0_sb[:, 0:2, :], in_=xT[:, 0, 0:2, :])
            nc.sync.dma_start(out=x0_sb[:, 2:4, :], in_=xT[:, 0, 2:4, :])
            nc.gpsimd.dma_start(out=x0_sb[:, 4:6, :], in_=xT[:, 0, 4:6, :])
            nc.gpsimd.dma_start(out=x0_sb[:, 6:8, :], in_=xT[:, 0, 6:8, :])
            nc.sync.dma_start(out=sin_sb[:, 0:TC], in_=sin_e[:, 0:TC])
            nc.gpsimd.dma_start(out=cos_sb[:, 0:TC], in_=cos_e[:, 0:TC])

            ident = const.tile([128, 128], f16)
            make_identity(nc, ident)
            # mask2[k, h, q] = 1 where q >= k (keep), both heads
            mask2 = const.tile([128, 2, 128], f16)
            make_upper_triangular(nc, mask2[:, 0, :], val=1.0, diag=True)
            make_upper_triangular(nc, mask2[:, 1, :], val=1.0, diag=True)
            # row of ones: lhsT of the reciprocal-broadcast matmul
            ones64 = const.tile([1, 64], f16)
            nc.gpsimd.memset(ones64, 1.0)

            # PE warmup: dependency-free matmuls on the on-chip identity
            # bridge the PE HAM activity window until the first x data
            # lands (~9.5us), so real matmuls start at 2.4 GHz.
            warm_ps = work.tile([128, 128], f32, tag="work", name="warm")
            for _ in range(18):
                nc.tensor.matmul(warm_ps, ident, ident,
                                 start=True, stop=True)

            # ---- persistent tensors ----
            QrotT = persist.tile([128, B, T], f16)
            KrotT = persist.tile([128, B, T], f16)
            # V in t-major, per (batch, k-block): [V_A(64) | ones | V_B(64) | ones]
            Vaug = persist.tile([128, B, NKB, 130], f16)
            Yn = persist.tile([128, B, T], f16)
            ones_cols = Vaug.rearrange(
                "p b J (h x) -> p b J h x", x=65)[:, :, :, :, 64]
            nc.gpsimd.memset(ones_cols, 1.0)

            # ============ phase-1: x DMA (separate) + qkv compute ==========
            def dma_x(ci):
                """Issue chunk ci's x load, split across sync+gpsimd."""
                x_sb = xp.tile([128, CCH, TC], f16, tag="x")
                nc.sync.dma_start(out=x_sb[:, 0:4, :],
                                  in_=xT[:, ci, 0:4, :])
                nc.gpsimd.dma_start(out=x_sb[:, 4:8, :],
                                    in_=xT[:, ci, 4:8, :])
                return x_sb

            x_tiles = {}

            def ph1_compute(b, i, swp_pool=None):
                """Emission closures for qkv+rotary of t-chunk i, batch b."""
                ci = b * NC_ + i
                cs = slice(i * TC, (i + 1) * TC)
                state = {}
                spool = swp_pool or work
                stag = "st" if swp_pool is not None else "work"

                def qk_group(g):
                    x_sb = x_tiles[ci]
                    dst = QrotT if g == 0 else KrotT
                    acc = work.tile([128, TC], f32, tag="work", name="acc")
                    for cc in range(CCH):
                        nc.tensor.matmul(
                            acc, wqkv_sb[:, g, cc, :], x_sb[:, cc, :],
                            start=(cc == 0), stop=(cc == CCH - 1))
                    graw = rotp.tile([128, TC], f16, tag="graw")
                    nc.vector.tensor_copy(graw, acc)
                    swp = spool.tile([128, TC], f32, tag=stag, name="swp")
                    nc.tensor.matmul(swp, pswap_sb, graw,
                                     start=True, stop=True)
                    t1 = rotp.tile([128, TC], f16, tag="t1")
                    nc.vector.tensor_mul(t1, graw, cos_sb[:, cs])
                    t2 = rotp.tile([128, TC], f16, tag="t2")
                    nc.vector.tensor_mul(t2, swp, sin_sb[:, cs])
                    nc.vector.tensor_add(dst[:, b, cs], t1, t2)

                def v_group():
                    x_sb = x_tiles[ci]
                    acc = work.tile([128, TC], f32, tag="work", name="vacc")
                    for cc in range(CCH):
                        nc.tensor.matmul(
                            acc, wqkv_sb[:, 2, cc, :], x_sb[:, cc, :],
                            start=(cc == 0), stop=(cc == CCH - 1))
                    vtmp = rotp.tile([128, TC], f16, tag="vtmp")
                    nc.vector.tensor_copy(vtmp, acc)
                    state["vtmp"] = vtmp

                def v_trans():
                    vtmp = state["vtmp"]
                    for q in range(TC // 128):
                        J = i * (TC // 128) + q
                        vt = work.tile([128, 128], f16, tag="work", name="vt")
                        nc.tensor.transpose(
                            vt, vtmp[:, q * 128:(q + 1) * 128], ident)
                        vdst = Vaug[:, b, J, :].rearrange(
                            "p (h x) -> p h x", x=65)[:, :, 0:64]
                        vsrc = vt.rearrange("p (h x) -> p h x", h=2)
                        nc.vector.tensor_copy(vdst, vsrc)

                return [lambda: qk_group(0), lambda: qk_group(1),
                        v_group, v_trans]

            # ============ phase-2 unit: attention for (b, q-window) ========
            def emit_unit(b, qs, W, next_pieces, last, n_act_pout):
                """Attention + normalize + proj for queries [qs, qs+W)."""
                jmax = (qs + W) // KB - 1
                # for the tail halves, process diagonal (masked) blocks
                # first so the final scores->exp->mask->PV chain is a
                # mask-free full block
                js = list(range(jmax + 1))
                pts = {}
                pieces_done = 0
                # tiles are allocated full-width (uniform slot sizes per
                # pool tag) and sliced to W
                ypss = [acc2.tile([128, TC], f32, tag="acc2", name="yps")
                        for _ in range(2)]

                def pv(j):
                    pt, off = pts[j]
                    for h in range(2):
                        nc.tensor.matmul(
                            ypss[h][0:65, off:W],
                            Vaug[:, b, j, h * 65:(h + 1) * 65],
                            pt[:, h, off:W],
                            start=(j == js[0]), stop=(j == js[-1]))

                for ji, j in enumerate(js):
                    off = max(0, j * KB - qs)  # valid col offset in window
                    stf = stp.tile([128, 2, TC], f32, tag="st", name="st")
                    st = stf[:, :, 0:W]
                    for h in range(2):
                        hs = slice(h * 64, (h + 1) * 64)
                        nc.tensor.matmul(
                            st[:, h, off:W],
                            KrotT[hs, b, j * KB:(j + 1) * KB],
                            QrotT[hs, b, qs + off:qs + W],
                            start=True, stop=True)
                    ptf = ptp.tile([128, 2, TC], f16, tag="pt", name="pt")
                    pt = ptf[:, :, 0:W]
                    nc.scalar.activation(
                        pt[:, :, off:W], st[:, :, off:W],
                        mybir.ActivationFunctionType.Exp)
                    if j * KB >= qs:  # diagonal band: triangular mask
                        nc.vector.tensor_mul(
                            pt[:, :, off:off + 128],
                            pt[:, :, off:off + 128], mask2)
                    pts[j] = (pt, off)
                    # PV trails scores by 2 so exp/mask have drained
                    if ji >= 2:
                        pv(js[ji - 2])
                    # interleave next chunk's qkv work into the PE stream
                    want = (len(next_pieces) * (ji + 1)) // (jmax + 1)
                    while pieces_done < want:
                        next_pieces[pieces_done]()
                        pieces_done += 1
                while pieces_done < len(next_pieces):
                    next_pieces[pieces_done]()
                    pieces_done += 1
                pv(js[-2])
                pv(js[-1])
                if last == 2:
                    # tail filler: keep the PE HAM-warm through the final
                    # half-unit's normalize chain
                    tail_ps = work.tile([128, 128], f32, tag="work",
                                        name="tail_ps")
                    for _ in range(16):
                        nc.tensor.matmul(tail_ps, ident, ident,
                                         start=True, stop=True)

                # ---- normalize: rows 0-63 divided by the ones-row (64) ----
                # custom-DVE reciprocal misreads PSUM/cross-partition inputs,
                # so stage both heads' denominators into SBUF partition 0.
                dsbf = normp.tile([1, 2, TC], f32, tag="dsb")
                dsb = dsbf[:, :, 0:W]
                for h in range(2):
                    nc.vector.tensor_copy(dsb[0:1, h, :],
                                          ypss[h][64:65, 0:W])
                recf = normp.tile([1, 2, TC], f32, tag="rec")
                rec = recf[:, :, 0:W]
                for h in range(2):
                    nc.vector.reciprocal_approx_fast(
                        out=rec[0:1, h, :], in_=dsb[0:1, h, :])
                bcf = normp.tile([64, 2, TC], f32, tag="bc", name="bc")
                bc = bcf[:, :, 0:W]
                for h in range(2):
                    nc.gpsimd.partition_broadcast(bc[:, h, :], rec[0:1, h, :])
                cslice = slice(qs, qs + W)
                nc.vector.tensor_tensor(
                    out=Yn[0:64, b, cslice],
                    in0=ypss[0][0:64, 0:W], in1=bc[:, 0, :],
                    op=mybir.AluOpType.mult)
                ytmpf = normp.tile([64, TC], f16, tag="ytmp")
                ytmp = ytmpf[:, 0:W]
                nc.vector.tensor_tensor(
                    out=ytmp, in0=ypss[1][0:64, 0:W], in1=bc[:, 1, :],
                    op=mybir.AluOpType.mult)
                if not last:
                    # cross-partition move 0-63 -> 64-127 via DMA
                    nc.scalar.dma_start(out=Yn[64:128, b, cslice], in_=ytmp)

                # ---- projection for this q-window ----
                r0 = b * T + qs
                ntt = W // 128
                youtf = yp.tile([128, TC // 128, C], f16, tag="yo",
                                name="yout")
                yout = youtf[:, 0:ntt, :]
                for tt in range(ntt):
                    ts = slice(qs + tt * 128, qs + (tt + 1) * 128)
                    for half in range(2):
                        hw = slice(half * 512, (half + 1) * 512)
                        pout = acc2.tile([128, 512], f32, tag="acc2",
                                         name="pout")
                        if last:
                            # head B read straight from ytmp (partitions
                            # 0-63): no cross-partition YnB DMA in the
                            # tail chain, at the cost of a 2nd K=64 pass
                            nc.tensor.matmul(
                                pout, Yn[0:64, b, ts], wp_sb[0:64, hw],
                                start=True, stop=False)
                            nc.tensor.matmul(
                                pout, ytmp[:, tt * 128:(tt + 1) * 128],
                                wpB_sb[:, hw],
                                start=False, stop=True)
                        else:
                            nc.tensor.matmul(
                                pout, Yn[:, b, ts], wp_sb[:, hw],
                                start=True, stop=True)
                        dstap = yout[:, tt, half * 512:(half + 1) * 512]
                        if last == 2:
                            # final half: split each copy across ACT+DVE
                            # so the drain isn't paced by one engine
                            nc.scalar.copy(dstap[:, 0:256], pout[:, 0:256])
                            nc.vector.tensor_copy(dstap[:, 256:512],
                                                  pout[:, 256:512])
                        elif (tt * 2 + half) >= 2 * ntt - n_act_pout:
                            nc.scalar.copy(dstap, pout)
                        else:
                            nc.vector.tensor_copy(dstap, pout)
                    if last:
                        eng = (nc.gpsimd, nc.sync, nc.scalar, nc.sync)[tt % 4]
                        eng.dma_start(
                            out=y[r0 + tt * 128:r0 + (tt + 1) * 128, :],
                            in_=yout[:, tt, :])
                if not last:
                    yeng = nc.gpsimd if (qs // TC + b) % 2 == 0 else nc.scalar
                    yeng.dma_start(
                        out=y[r0:r0 + W, :].rearrange(
                            "(tt p) c -> p tt c", p=128),
                        in_=yout)

            # ================= emission: pipelined units ===================
            # units: chunks 0-6 full-width; chunk 7 as two 256 halves so
            # the tail chain pipelines.
            # each entry: (b, qs, W); chunk feeds are assigned to units in
            # order: unit k issues dma_x(k+2) and computes chunk k+1 where
            # k counts FULL-chunk positions (splits share their chunk's k).
            split_ci = 3 + NC_            # chunk index to emit as 2 halves
            units = []
            for ci in range(8):
                bb, ii = ci // NC_, ci % NC_
                if ci == split_ci:
                    units.append((bb, ii * TC, 256, ci))
                    units.append((bb, ii * TC + 256, 256, ci))
                else:
                    units.append((bb, ii * TC, TC, ci))

            x_tiles[0] = x0_sb
            x_tiles[1] = dma_x(1)
            x_tiles[2] = dma_x(2)
            # dummy broadcast AFTER the x prefetch issues: forces the
            # gpsimd custom-op library load to start now (its TDRAM DMA
            # rides behind x0-x2 on the ring, done ~24us) without
            # blocking any x-chunk DMA issue; unit 0's first real
            # broadcast (~31us) then never stalls on the load.
            libwarm = const.tile([64, 8], f32)
            nc.gpsimd.partition_broadcast(libwarm, libwarm[0:1, :])
            # chunk 0: Q/K groups now (swp staged via the idle stp pool so
            # the K chain isn't WAR-serialized on the 2-slot work ring);
            # V + transpose deferred into unit 0's feed.
            p0 = ph1_compute(0, 0, swp_pool=stp)
            p0[0]()
            p0[1]()
            fed_dma = {0, 1, 2}
            fed_cmp = {0}
            extra0 = [p0[2], p0[3]]
            for u, (b, qs, W, ci) in enumerate(units):
                nxt = list(extra0)
                extra0 = []
                if ci + 3 <= 7 and ci + 3 not in fed_dma:
                    fed_dma.add(ci + 3)
                    nxt.append(lambda c3=ci + 3: x_tiles.__setitem__(
                        c3, dma_x(c3)))
                if ci + 1 <= 7 and ci + 1 not in fed_cmp:
                    fed_cmp.add(ci + 1)
                    nxt.extend(ph1_compute((ci + 1) // NC_, (ci + 1) % NC_))
                last = 2 if u == len(units) - 1 else 0
                if ci == split_ci:
                    last = max(last, 1)
                n_act = (2 if last else
                         {0: 4, 1: 2, 2: 0, 3: 0}[min(qs // TC, 3)])
                emit_unit(b, qs, W, nxt, last, n_act)

    nc.finalize()
    return nc


def _host_prep(x, cos, sin, w_attn, b_attn, w_proj):
    """Shared + per-core input arrays (all fp16 except noted)."""
    x2 = np.asarray(x, dtype=np.float32).reshape(BT, C)
    xT16 = np.ascontiguousarray(x2.T).astype(np.float16)
    # repack [C, BT] -> [p, chunk, cc, t] so each DMA chunk reads one
    # contiguous 8KB run per partition
    xprep = np.ascontiguousarray(
        xT16.reshape(CCH, 128, B * NC_, TC).transpose(1, 2, 0, 3))

    cos = np.asarray(cos, dtype=np.float32)
    sin = np.asarray(sin, dtype=np.float32)
    d = np.arange(128) % 64
    freq_i = d // 2
    sign = np.where(d % 2 == 0, -1.0, 1.0).astype(np.float32)
    cos_exp = cos[:, freq_i].T.astype(np.float16)               # [128, T]
    sin_exp = (sign[:, None] * sin[:, freq_i].T).astype(np.float16)

    pswap = np.zeros((128, 128), dtype=np.float16)
    idx = np.arange(128)
    pswap[idx ^ 1, idx] = 1.0

    w_attn = np.asarray(w_attn, dtype=np.float32)
    w_proj = np.asarray(w_proj, dtype=np.float32)
    scale = 1.0 / np.sqrt(HD)

    per_core = []
    for m in range(N_CORES):
        groups = []
        for g in range(3):          # q, k, v blocks of w_attn
            cols = []
            for hh in range(HPC):
                hglob = m * HPC + hh
                blk = w_attn[:, g * C + hglob * HD:(g * C + (hglob + 1) * HD)]
                if g == 0:
                    blk = blk * scale
                cols.append(blk)
            gs = np.concatenate(cols, axis=1).astype(np.float16)  # [C, 128]
            # [C, 128] -> [p, cc, j] (contiguous 2KB per partition)
            groups.append(gs.reshape(CCH, 128, 128).transpose(1, 0, 2))
        w_stack = np.ascontiguousarray(np.stack(groups, axis=1))
        wp_m = w_proj[m * HPC * HD:(m + 1) * HPC * HD, :].astype(np.float16)
        per_core.append((w_stack, wp_m))
    return xprep, cos_exp, sin_exp, pswap, per_core


def kernel(x, cos, sin, w_attn, b_attn, w_proj, b_proj):
    from concourse.bass_utils import run_bass_kernel_spmd

    b_attn = np.asarray(b_attn, dtype=np.float32)
    assert not np.any(b_attn), "nonzero b_attn not supported by this kernel"

    xT16, cos_exp, sin_exp, pswap, per_core = _host_prep(
        x, cos, sin, w_attn, b_attn, w_proj)

    if "nc" not in _CACHE:
        _CACHE["nc"] = _build_bass()
    nc = _CACHE["nc"]

    in_maps = []
    for m in range(N_CORES):
        w_stack, wp_m = per_core[m]
        in_maps.append({
            "xT": xT16, "wqkv": w_stack, "wp": wp_m,
            "cos_e": cos_exp, "sin_e": sin_exp, "pswap": pswap,
        })

    res = run_bass_kernel_spmd(nc, in_maps, core_ids=list(range(N_CORES)))
    _CACHE["last_result"] = res

    y = np.zeros((BT, C), dtype=np.float64)
    for m in range(N_CORES):
        y += res.results[m]["y"].astype(np.float64)
    y = y + np.asarray(b_proj, dtype=np.float64)[None, :]
    return y.reshape(B, T, C).astype(np.float32)


# revision 66
# speedup vs baseline: 1.2311x; 1.0245x over previous
"""Causal self-attention with rotary embeddings on 8 Trainium2 NeuronCores.

Tensor-parallel over heads: 16 heads / 8 cores = 2 heads per core.
Each core computes qkv for its 2 heads, rotary, causal attention, and a
partial output projection (its 128 rows of w_proj); the host sums the 8
partial outputs.

Device-side structure (per core, heads A/B local):
  - Everything "transposed": Q^T/K^T stored [d(128=A:0-63,B:64-127), t].
  - Work is emitted as a pipeline of (batch, q-chunk) units:
      scores+exp(unit) | qkv+rotary(next chunk) | PV | normalize | proj
    so TensorE always has dense matmul work while ScalarE exponentiates
    and the projection + y DMA spread across the whole kernel.
  - Scores S^T = K_blk @ Q^T -> [k(128), q], computed for both heads as
    two row-tiled matmuls (head A rows 0-63, head B rows 64-127) that
    run concurrently in the PE array (K=64 each).
  - exp per k-block over both heads in one ACTIVATE on [128, 2, W-off]
    (off = causal column offset); diagonal 128-col block masked via one
    GpSimd multiply (2-head mask const); PV matmuls use partial-N
    accumulation so fully-masked columns are never touched.
  - Softmax denominator via a ones-augmented V column (extra lhsT column
    produces the k-sum row). No max-subtraction (scores are O(6)).
  - Rotary applied in the transposed layout via a pair-swap permutation
    matmul: rot(q) = cos_exp * q + sin_sgn * (Pswap @ q).
  - V transposed to t-major [k, d] tiles with the PE transpose path.
  - The last unit is split into two 256-wide half-units so its
    normalize->proj->DMA tail pipelines instead of serializing.

DMA layout: three rings (sync HWDGE, scalar HWDGE, gpsimd SWDGE q0).
x chunks are split half/half across sync+gpsimd and prefetched one unit
early; startup constants are wave-ordered by first-use time so the first
QKV matmuls start ~9us in and the PE never idles past the HAM window.

All matmul inputs fp16 (1 cyc/row on PE); accumulation fp32 in PSUM.
"""

import numpy as np

B, T, C, H = 2, 2048, 1024, 16
HD = C // H            # 64
N_CORES = 8
HPC = H // N_CORES     # 2 heads per core
BT = B * T             # 4096
TC = 512               # t-chunk size for qkv phase
NC_ = T // TC          # 4 chunks per batch
KB = 128               # k-block size
NKB = T // KB          # 16 k-blocks per batch
CCH = C // 128         # 8 contraction chunks

_CACHE = {}


def _build_bass():
    import concourse.bacc as bacc
    import concourse.mybir as mybir
    import concourse.tile as tile
    from concourse.masks import make_identity, make_upper_triangular

    f16 = mybir.dt.float16
    f32 = mybir.dt.float32

    nc = bacc.Bacc(num_swdge_queues=4)

    # host-prepacked layouts: per-partition-contiguous so DMA bursts are
    # large (8KB x chunks, 2KB wqkv group slices) instead of sub-KB rows
    xT = nc.dram_tensor("xT", [128, NC_ * B, CCH, TC], f16,
                        kind="ExternalInput")
    wqkv = nc.dram_tensor("wqkv", [128, 3, CCH, 128], f16,
                          kind="ExternalInput")
    wp = nc.dram_tensor("wp", [HPC * HD, C], f16, kind="ExternalInput")
    cos_e = nc.dram_tensor("cos_e", [128, T], f16, kind="ExternalInput")
    sin_e = nc.dram_tensor("sin_e", [128, T], f16, kind="ExternalInput")
    pswap = nc.dram_tensor("pswap", [128, 128], f16, kind="ExternalInput")
    y = nc.dram_tensor("y", [BT, C], f16, kind="ExternalOutput")

    with tile.TileContext(nc) as tc:
        with (
            tc.tile_pool(name="const", bufs=1) as const,
            tc.tile_pool(name="persist", bufs=1) as persist,
            tc.tile_pool(name="xp", bufs=4) as xp,
            tc.tile_pool(name="rot", bufs=3) as rotp,
            tc.tile_pool(name="ptp", bufs=24) as ptp,
            tc.tile_pool(name="np_", bufs=3) as normp,
            tc.tile_pool(name="yp", bufs=2) as yp,
            tc.tile_pool(name="work", bufs=2, space="PSUM") as work,
            tc.tile_pool(name="acc2", bufs=2, space="PSUM") as acc2,
            tc.tile_pool(name="stp", bufs=2, space="PSUM") as stp,
        ):
            # ---- constants: wave-ordered startup DMAs across 3 rings ----
            # scalar ring carries the weights (ordered by first use);
            # sync/gpsimd carry the first x chunk + first cos/sin columns.
            wqkv_sb = const.tile([128, 3, CCH, 128], f16)
            cos_sb = const.tile([128, T], f16)
            sin_sb = const.tile([128, T], f16)
            pswap_sb = const.tile([128, 128], f16)
            wp_sb = const.tile([128, C], f16)

            nc.scalar.dma_start(out=wqkv_sb[:, 0], in_=wqkv[:, 0])  # Q
            nc.scalar.dma_start(out=wqkv_sb[:, 1], in_=wqkv[:, 1])  # K
            nc.scalar.dma_start(out=pswap_sb, in_=pswap[:, :])
            nc.scalar.dma_start(out=wqkv_sb[:, 2], in_=wqkv[:, 2])  # V
            wpB_sb = const.tile([64, C], f16)
            gate_sb = const.tile([1, 8], f16)

            # x chunk 0 before anything else on the sync/gpsimd rings
            # (ring FIFO = priority), then the chunk-0 cos/sin columns,
            # then the chunk-1 prefetch rides behind them.
            x0_sb = xp.tile([128, CCH, TC], f16, tag="x")
            nc.sync.dma_start(out=x0_sb[:, 0:2, :], in_=xT[:, 0, 0:2, :])
            nc.sync.dma_start(out=x0_sb[:, 2:4, :], in_=xT[:, 0, 2:4, :])
            nc.gpsimd.dma_start(out=x0_sb[:, 4:6, :], in_=xT[:, 0, 4:6, :])
            nc.gpsimd.dma_start(out=x0_sb[:, 6:8, :], in_=xT[:, 0, 6:8, :])
            nc.sync.dma_start(out=sin_sb[:, 0:TC], in_=sin_e[:, 0:TC])
            nc.gpsimd.dma_start(out=cos_sb[:, 0:TC], in_=cos_e[:, 0:TC])

            ident = const.tile([128, 128], f16)
            make_identity(nc, ident)
            # mask2[k, h, q] = 1 where q >= k (keep), both heads
            mask2 = const.tile([128, 2, 128], f16)
            make_upper_triangular(nc, mask2[:, 0, :], val=1.0, diag=True)
            make_upper_triangular(nc, mask2[:, 1, :], val=1.0, diag=True)


            # PE warmup: dependency-free matmuls on the on-chip identity
            # bridge the PE HAM activity window until the first x data
            # lands (~9.5us), so real matmuls start at 2.4 GHz.
            warm_ps = work.tile([128, 128], f32, tag="work", name="warm")
            for _ in range(18):
                nc.tensor.matmul(warm_ps, ident, ident,
                                 start=True, stop=True)

            # ---- persistent tensors ----
            QrotT = persist.tile([128, B, T], f16)
            KrotT = persist.tile([128, B, T], f16)
            # V in t-major, per (batch, k-block): [V_A(64) | ones | V_B(64) | ones]
            Vaug = persist.tile([128, B, NKB, 130], f16)
            Yn = persist.tile([128, B, T], f16)
            ones_cols = Vaug.rearrange(
                "p b J (h x) -> p b J h x", x=65)[:, :, :, :, 64]
            nc.gpsimd.memset(ones_cols, 1.0)

            # ============ phase-1: x DMA (separate) + qkv compute ==========
            def dma_x(ci):
                """Issue chunk ci's x load, split across sync+gpsimd."""
                x_sb = xp.tile([128, CCH, TC], f16, tag="x")
                nc.sync.dma_start(out=x_sb[:, 0:4, :],
                                  in_=xT[:, ci, 0:4, :])
                nc.gpsimd.dma_start(out=x_sb[:, 4:8, :],
                                    in_=xT[:, ci, 4:8, :])
                return x_sb

            x_tiles = {}

            def ph1_compute(b, i, swp_pool=None):
                """Emission closures for qkv+rotary of t-chunk i, batch b."""
                ci = b * NC_ + i
                cs = slice(i * TC, (i + 1) * TC)
                state = {}
                spool = swp_pool or work
                stag = "st" if swp_pool is not None else "work"

                def qk_group(g):
                    x_sb = x_tiles[ci]
                    dst = QrotT if g == 0 else KrotT
                    acc = work.tile([128, TC], f32, tag="work", name="acc")
                    for cc in range(CCH):
                        nc.tensor.matmul(
                            acc, wqkv_sb[:, g, cc, :], x_sb[:, cc, :],
                            start=(cc == 0), stop=(cc == CCH - 1))
                    graw = rotp.tile([128, TC], f16, tag="graw")
                    nc.vector.tensor_copy(graw, acc)
                    swp = spool.tile([128, TC], f32, tag=stag, name="swp")
                    nc.tensor.matmul(swp, pswap_sb, graw,
                                     start=True, stop=True)
                    t1 = rotp.tile([128, TC], f16, tag="t1")
                    nc.vector.tensor_mul(t1, graw, cos_sb[:, cs])
                    t2 = rotp.tile([128, TC], f16, tag="t2")
                    nc.vector.tensor_mul(t2, swp, sin_sb[:, cs])
                    nc.vector.tensor_add(dst[:, b, cs], t1, t2)

                def v_group():
                    x_sb = x_tiles[ci]
                    acc = work.tile([128, TC], f32, tag="work", name="vacc")
                    for cc in range(CCH):
                        nc.tensor.matmul(
                            acc, wqkv_sb[:, 2, cc, :], x_sb[:, cc, :],
                            start=(cc == 0), stop=(cc == CCH - 1))
                    vtmp = rotp.tile([128, TC], f16, tag="vtmp")
                    nc.vector.tensor_copy(vtmp, acc)
                    state["vtmp"] = vtmp

                def v_trans():
                    vtmp = state["vtmp"]
                    for q in range(TC // 128):
                        J = i * (TC // 128) + q
                        vt = work.tile([128, 128], f16, tag="work", name="vt")
                        nc.tensor.transpose(
                            vt, vtmp[:, q * 128:(q + 1) * 128], ident)
                        vdst = Vaug[:, b, J, :].rearrange(
                            "p (h x) -> p h x", x=65)[:, :, 0:64]
                        vsrc = vt.rearrange("p (h x) -> p h x", h=2)
                        nc.vector.tensor_copy(vdst, vsrc)

                return [lambda: qk_group(0), lambda: qk_group(1),
                        v_group, v_trans]

            # ============ phase-2 unit: attention for (b, q-window) ========
            def emit_unit(b, qs, W, next_pieces, last, n_act_pout):
                """Attention + normalize + proj for queries [qs, qs+W)."""
                jmax = (qs + W) // KB - 1
                # for the tail halves, process diagonal (masked) blocks
                # first so the final scores->exp->mask->PV chain is a
                # mask-free full block
                js = list(range(jmax + 1))
                pts = {}
                pieces_done = 0
                # tiles are allocated full-width (uniform slot sizes per
                # pool tag) and sliced to W
                ypss = [acc2.tile([128, TC], f32, tag="acc2", name="yps")
                        for _ in range(2)]

                def pv(j):
                    pt, off = pts[j]
                    for h in range(2):
                        nc.tensor.matmul(
                            ypss[h][0:65, off:W],
                            Vaug[:, b, j, h * 65:(h + 1) * 65],
                            pt[:, h, off:W],
                            start=(j == js[0]), stop=(j == js[-1]))

                for ji, j in enumerate(js):
                    off = max(0, j * KB - qs)  # valid col offset in window
                    stf = stp.tile([128, 2, TC], f32, tag="st", name="st")
                    st = stf[:, :, 0:W]
                    for h in range(2):
                        hs = slice(h * 64, (h + 1) * 64)
                        nc.tensor.matmul(
                            st[:, h, off:W],
                            KrotT[hs, b, j * KB:(j + 1) * KB],
                            QrotT[hs, b, qs + off:qs + W],
                            start=True, stop=True)
                    ptf = ptp.tile([128, 2, TC], f16, tag="pt", name="pt")
                    pt = ptf[:, :, 0:W]
                    nc.scalar.activation(
                        pt[:, :, off:W], st[:, :, off:W],
                        mybir.ActivationFunctionType.Exp)
                    if j * KB >= qs:  # diagonal band: triangular mask
                        # (NB: gpsimd.tensor_mul would thrash the gpsimd
                        # ucode library against partition_broadcast)
                        nc.vector.tensor_mul(
                            pt[:, :, off:off + 128],
                            pt[:, :, off:off + 128], mask2)
                    pts[j] = (pt, off)
                    # PV trails scores by 2 so exp/mask have drained
                    if ji >= 2:
                        pv(js[ji - 2])
                    # interleave next chunk's qkv work into the PE stream;
                    # in large units finish the pieces a few blocks early
                    # so the next unit's rotary outputs are ready the
                    # moment this unit's attention drains
                    denom = (jmax - 3) if jmax >= 8 else (jmax + 1)
                    want = min(len(next_pieces),
                               (len(next_pieces) * (ji + 1)) // denom)
                    while pieces_done < want:
                        next_pieces[pieces_done]()
                        pieces_done += 1
                while pieces_done < len(next_pieces):
                    next_pieces[pieces_done]()
                    pieces_done += 1
                pv(js[-2])
                pv(js[-1])
                if last == 2:
                    # tail filler: keep the PE HAM-warm through the final
                    # half-unit's normalize chain
                    tail_ps = work.tile([128, 128], f32, tag="work",
                                        name="tail_ps")
                    for _ in range(16):
                        nc.tensor.matmul(tail_ps, ident, ident,
                                         start=True, stop=True)

                # ---- normalize: rows 0-63 divided by the ones-row (64) ----
                # custom-DVE reciprocal misreads PSUM/cross-partition inputs,
                # so stage both heads' denominators into SBUF partition 0.
                dsbf = normp.tile([1, 2, TC], f32, tag="dsb")
                dsb = dsbf[:, :, 0:W]
                for h in range(2):
                    nc.vector.tensor_copy(dsb[0:1, h, :],
                                          ypss[h][64:65, 0:W])
                recf = normp.tile([1, 2, TC], f32, tag="rec")
                rec = recf[:, :, 0:W]
                if W == TC:
                    nc.vector.reciprocal_approx_fast(
                        out=recf.rearrange("p a x -> p (a x)"),
                        in_=dsbf.rearrange("p a x -> p (a x)"))
                else:
                    for h in range(2):
                        nc.vector.reciprocal_approx_fast(
                            out=rec[0:1, h, :], in_=dsb[0:1, h, :])
                bcf = normp.tile([64, 2, TC], f32, tag="bc", name="bc")
                bc = bcf[:, :, 0:W]
                for h in range(2):
                    nc.gpsimd.partition_broadcast(bc[:, h, :], rec[0:1, h, :])
                cslice = slice(qs, qs + W)
                nc.vector.tensor_tensor(
                    out=Yn[0:64, b, cslice],
                    in0=ypss[0][0:64, 0:W], in1=bc[:, 0, :],
                    op=mybir.AluOpType.mult)
                ytmpf = normp.tile([64, TC], f16, tag="ytmp")
                ytmp = ytmpf[:, 0:W]
                nc.vector.tensor_tensor(
                    out=ytmp, in0=ypss[1][0:64, 0:W], in1=bc[:, 1, :],
                    op=mybir.AluOpType.mult)
                if not last:
                    # cross-partition move 0-63 -> 64-127 via DMA
                    nc.scalar.dma_start(out=Yn[64:128, b, cslice], in_=ytmp)

                # ---- projection for this q-window ----
                r0 = b * T + qs
                ntt = W // 128
                youtf = yp.tile([128, TC // 128, C], f16, tag="yo",
                                name="yout")
                yout = youtf[:, 0:ntt, :]
                for tt in range(ntt):
                    ts = slice(qs + tt * 128, qs + (tt + 1) * 128)
                    for half in range(2):
                        hw = slice(half * 512, (half + 1) * 512)
                        pout = acc2.tile([128, 512], f32, tag="acc2",
                                         name="pout")
                        if last:
                            # head B read straight from ytmp (partitions
                            # 0-63): no cross-partition YnB DMA in the
                            # tail chain, at the cost of a 2nd K=64 pass
                            nc.tensor.matmul(
                                pout, Yn[0:64, b, ts], wp_sb[0:64, hw],
                                start=True, stop=False)
                            nc.tensor.matmul(
                                pout, ytmp[:, tt * 128:(tt + 1) * 128],
                                wpB_sb[:, hw],
                                start=False, stop=True)
                        else:
                            nc.tensor.matmul(
                                pout, Yn[:, b, ts], wp_sb[:, hw],
                                start=True, stop=True)
                        dstap = yout[:, tt, half * 512:(half + 1) * 512]
                        if last == 2:
                            # final half: split each copy across ACT+DVE
                            # so the drain isn't paced by one engine
                            nc.scalar.copy(dstap[:, 0:256], pout[:, 0:256])
                            nc.vector.tensor_copy(dstap[:, 256:512],
                                                  pout[:, 256:512])
                        elif (tt * 2 + half) >= 2 * ntt - n_act_pout:
                            nc.scalar.copy(dstap, pout)
                        else:
                            nc.vector.tensor_copy(dstap, pout)
                    if last:
                        eng = (nc.gpsimd, nc.sync, nc.scalar, nc.sync)[tt % 4]
                        eng.dma_start(
                            out=y[r0 + tt * 128:r0 + (tt + 1) * 128, :],
                            in_=yout[:, tt, :])
                if not last:
                    yeng = nc.gpsimd if (qs // TC + b) % 2 == 0 else nc.scalar
                    yeng.dma_start(
                        out=y[r0:r0 + W, :].rearrange(
                            "(tt p) c -> p tt c", p=128),
                        in_=yout)

            # ================= emission: pipelined units ===================
            # units: chunks 0-6 full-width; chunk 7 as two 256 halves so
            # the tail chain pipelines.
            # each entry: (b, qs, W); chunk feeds are assigned to units in
            # order: unit k issues dma_x(k+2) and computes chunk k+1 where
            # k counts FULL-chunk positions (splits share their chunk's k).
            split_ci = 3 + NC_            # chunk index to emit as 2 halves
            units = []
            for ci in range(8):
                bb, ii = ci // NC_, ci % NC_
                if ci == split_ci:
                    units.append((bb, ii * TC, 256, ci))
                    units.append((bb, ii * TC + 256, 256, ci))
                else:
                    units.append((bb, ii * TC, TC, ci))

            x_tiles[0] = x0_sb
            x_tiles[1] = dma_x(1)
            x_tiles[2] = dma_x(2)
            # dummy broadcast AFTER the x prefetch issues: forces the
            # gpsimd custom-op library load to start now (its TDRAM DMA
            # rides behind x0-x2 on the ring, done ~24us) without
            # blocking any x-chunk DMA issue; unit 0's first real
            # broadcast (~31us) then never stalls on the load.
            libwarm = const.tile([64, 8], f32)
            nc.gpsimd.partition_broadcast(libwarm, libwarm[0:1, :])
            # chunk 0: Q/K groups now (swp staged via the idle stp pool so
            # the K chain isn't WAR-serialized on the 2-slot work ring);
            # V + transpose deferred into unit 0's feed.
            p0 = ph1_compute(0, 0, swp_pool=stp)
            p0[0]()
            p0[1]()
            # late-wave weights: gated behind chunk 0's rotary output so
            # their transfers don't steal HBM bandwidth from x0/x1 during
            # the startup-critical first ~12us. wp is needed at ~40us,
            # cos/sin cols 512+ at ~20us, wpB at ~170us.
            nc.scalar.copy(gate_sb, QrotT[0:1, 0, 0:8])
            nc.scalar.dma_start(out=cos_sb[:, TC:], in_=cos_e[:, TC:])
            nc.scalar.dma_start(out=sin_sb[:, TC:], in_=sin_e[:, TC:])
            nc.scalar.dma_start(out=wp_sb, in_=wp[:, :])
            # head-B projection rows re-staged at partitions 0-63 for the
            # tail half-units (lets their proj read ytmp directly and skip
            # the cross-partition YnB DMA)
            nc.scalar.dma_start(out=wpB_sb, in_=wp[64:128, :])
            fed_dma = {0, 1, 2}
            fed_cmp = {0}
            extra0 = [p0[2], p0[3]]
            for u, (b, qs, W, ci) in enumerate(units):
                nxt = list(extra0)
                extra0 = []
                if ci + 3 <= 7 and ci + 3 not in fed_dma:
                    fed_dma.add(ci + 3)
                    nxt.append(lambda c3=ci + 3: x_tiles.__setitem__(
                        c3, dma_x(c3)))
                if ci + 1 <= 7 and ci + 1 not in fed_cmp:
                    fed_cmp.add(ci + 1)
                    nxt.extend(ph1_compute((ci + 1) // NC_, (ci + 1) % NC_))
                last = 2 if u == len(units) - 1 else 0
                if ci == split_ci:
                    last = max(last, 1)
                n_act = (2 if last else
                         {0: 4, 1: 2, 2: 0, 3: 0}[min(qs // TC, 3)])
                emit_unit(b, qs, W, nxt, last, n_act)

    nc.finalize()
    return nc


def _host_prep(x, cos, sin, w_attn, b_attn, w_proj):
    """Shared + per-core input arrays (all fp16 except noted)."""
    x2 = np.asarray(x, dtype=np.float32).reshape(BT, C)
    xT16 = np.ascontiguousarray(x2.T).astype(np.float16)
    # repack [C, BT] -> [p, chunk, cc, t] so each DMA chunk reads one
    # contiguous 8KB run per partition
    xprep = np.ascontiguousarray(
        xT16.reshape(CCH, 128, B * NC_, TC).transpose(1, 2, 0, 3))

    cos = np.asarray(cos, dtype=np.float32)
    sin = np.asarray(sin, dtype=np.float32)
    d = np.arange(128) % 64
    freq_i = d // 2
    sign = np.where(d % 2 == 0, -1.0, 1.0).astype(np.float32)
    cos_exp = cos[:, freq_i].T.astype(np.float16)               # [128, T]
    sin_exp = (sign[:, None] * sin[:, freq_i].T).astype(np.float16)

    pswap = np.zeros((128, 128), dtype=np.float16)
    idx = np.arange(128)
    pswap[idx ^ 1, idx] = 1.0

    w_attn = np.asarray(w_attn, dtype=np.float32)
    w_proj = np.asarray(w_proj, dtype=np.float32)
    scale = 1.0 / np.sqrt(HD)

    per_core = []
    for m in range(N_CORES):
        groups = []
        for g in range(3):          # q, k, v blocks of w_attn
            cols = []
            for hh in range(HPC):
                hglob = m * HPC + hh
                blk = w_attn[:, g * C + hglob * HD:(g * C + (hglob + 1) * HD)]
                if g == 0:
                    blk = blk * scale
                cols.append(blk)
            gs = np.concatenate(cols, axis=1).astype(np.float16)  # [C, 128]
            # [C, 128] -> [p, cc, j] (contiguous 2KB per partition)
            groups.append(gs.reshape(CCH, 128, 128).transpose(1, 0, 2))
        w_stack = np.ascontiguousarray(np.stack(groups, axis=1))
        wp_m = w_proj[m * HPC * HD:(m + 1) * HPC * HD, :].astype(np.float16)
        per_core.append((w_stack, wp_m))
    return xprep, cos_exp, sin_exp, pswap, per_core


def kernel(x, cos, sin, w_attn, b_attn, w_proj, b_proj):
    from concourse.bass_utils import run_bass_kernel_spmd

    b_attn = np.asarray(b_attn, dtype=np.float32)
    assert not np.any(b_attn), "nonzero b_attn not supported by this kernel"

    xT16, cos_exp, sin_exp, pswap, per_core = _host_prep(
        x, cos, sin, w_attn, b_attn, w_proj)

    if "nc" not in _CACHE:
        _CACHE["nc"] = _build_bass()
    nc = _CACHE["nc"]

    in_maps = []
    for m in range(N_CORES):
        w_stack, wp_m = per_core[m]
        in_maps.append({
            "xT": xT16, "wqkv": w_stack, "wp": wp_m,
            "cos_e": cos_exp, "sin_e": sin_exp, "pswap": pswap,
        })

    res = run_bass_kernel_spmd(nc, in_maps, core_ids=list(range(N_CORES)))
    _CACHE["last_result"] = res

    y = np.zeros((BT, C), dtype=np.float64)
    for m in range(N_CORES):
        y += res.results[m]["y"].astype(np.float64)
    y = y + np.asarray(b_proj, dtype=np.float64)[None, :]
    return y.reshape(B, T, C).astype(np.float32)
